# revision 2
# baseline (speedup 1.0000x reference)
"""Trainium2 Bass kernel for DietConv2dV2: 3x3 conv (stride 1, pad 1) + bias.

x: (16, 8, 1024, 1024) fp32, weight: (8, 8, 3, 3), bias: (8,) -> out like x.

Strategy
--------
Data-parallel: 16 images / 8 cores = 2 images per core, no collectives.

Per core the conv runs as a banded matmul on the PE array over
NON-OVERLAPPING 16-row tiles, so every input row is read from HBM
exactly once (the previous 14-row-block design re-read 2/16 rows,
14% extra input traffic; this kernel is at the 2x64MiB/core
read+write roofline, ~358 GB/s per core):

  - tile t holds input rows [16t-1, 16t+15) as K = 16 rows x 8 ch = 128
    partitions (p = r*8 + ci, r = row - (16t-1)).
  - out block t = rows [16t, 16t+16), M = 16 rows x 8 ch = 128 columns.
  - out row ho needs input rows r = ho+kh (kh in 0..2).  r <= 15 taps
    come from tile t via 3 kw-tap matmuls (stationary band S_A[kw],
    [128 x 128]); the r = 16, 17 taps (out rows 14, 15 only) come from
    the FIRST 2 rows of tile t+1 via 3 more matmuls with K = 16
    (stationary S_B[kw], [16 x 128]), PSUM-accumulated into the same
    bank.  12 matmuls / 6+6 per 512-wide chunk; sustained MM cadence is
    ~232 ns (ldweights hides in the PE reorder window), ~2.8 us/block
    vs the 2.9 us/block DMA budget.
  - tile 64 (1 valid row + zeros) feeds block 63's boundary taps.

Inputs stream HBM->SBUF through the SWDGE cast path (fp32 -> float32r)
for the full-rate single-pass fp32r matmul (~1.5e-4 rel err).  Bias is
fused into the PSUM->SBUF eviction as a DVE per-partition add.  Output
DMAs alternate across both HWDGE rings; the last few blocks fan out
over sync/scalar/gpsimd to shorten the write-only tail.
"""

import numpy as np

import bass_rust
import concourse.bass as bass
import concourse.mybir as mybir
from concourse.tile import TileContext
from concourse.bass_utils import run_bass_kernel_spmd

F32 = mybir.dt.float32
F32R = mybir.dt.float32r

N_CORES = 8
IMG_PER_CORE = 2
C = 8          # channels (in == out)
H = 1024
W = 1024
KS = 3         # kernel size
HB = 16        # output rows per block == input rows per tile
M = C * HB     # 128 stationary columns
WCHUNK = 512   # PSUM bank = 512 fp32
XBUFS = 14
OBUFS = 8


def _split_excess_waits(nc):
    """This walrus build accepts 1 sync-wait per instruction (2 for
    EventSemaphore); Tile's final drain and ldweights can end up with
    more.  Move overflow waits onto EventSemaphore carriers inserted
    before the offender on the same engine."""
    for fn in nc.m.functions:
        for blk in fn.blocks:
            out = []
            changed = False
            for inst in blk.instructions:
                si = inst.sync_info
                cap = 2 if inst.opcode == "EventSemaphore" else 1
                waits = list(si.on_wait) if si is not None else []
                if len(waits) > cap:
                    changed = True
                    overflow, keep = waits[:-cap], waits[-cap:]
                    for j in range(0, len(overflow), 2):
                        es = mybir.InstEventSemaphore(
                            name=nc.get_next_instruction_name(), ins=[], outs=[]
                        )
                        es.engine = inst.engine
                        es.sync_info = bass_rust.SyncInfo(
                            on_wait=overflow[j : j + 2], on_update=[]
                        )
                        nc.register_instruction(es, overwrite=True)
                        out.append(es)
                    inst.sync_info = bass_rust.SyncInfo(
                        on_wait=keep, on_update=list(si.on_update)
                    )
                out.append(inst)
            if changed:
                blk.instructions = out


def _build(nimg, h, w, reps=1, salt=0):
    nblocks = h // HB            # 64
    nchunks = w // WCHUNK        # 2
    nall = nimg * nblocks

    nc = bass.Bass(name=f"dietconv_s{salt}")
    x = nc.dram_tensor("x", [nimg, C, h, w], F32, kind="ExternalInput")
    wa = nc.dram_tensor("wa", [KS, 128, M], F32, kind="ExternalInput")
    wb = nc.dram_tensor("wb", [KS, 16, M], F32, kind="ExternalInput")
    bv = nc.dram_tensor("biasv", [M, 1], F32, kind="ExternalInput")
    out = nc.dram_tensor("out", [nimg, C, h, w], F32, kind="ExternalOutput")

    # row-major (h, c) views so SBUF partition p = r*8 + ci
    xr = x.rearrange("n c h w -> n h c w")
    outr = out.rearrange("n c h w -> n h c w")

    with TileContext(nc) as tc:
        with (
            tc.tile_pool(name="wpool", bufs=1) as wpool,
            tc.tile_pool(name="xpool", bufs=XBUFS) as xpool,
            tc.tile_pool(name="opool", bufs=OBUFS) as opool,
            tc.tile_pool(name="pspool", bufs=4, space="PSUM") as pspool,
        ):
            wta = []
            wtb = []
            for kw in range(KS):
                t = wpool.tile([128, M], F32R, name=f"wa{kw}")
                nc.gpsimd.dma_start(out=t[:], in_=wa[kw])
                wta.append(t)
                t = wpool.tile([16, M], F32R, name=f"wb{kw}")
                nc.gpsimd.dma_start(out=t[:], in_=wb[kw])
                wtb.append(t)
            bt = wpool.tile([M, 1], F32, name="bt")
            nc.sync.dma_start(out=bt[:], in_=bv[:])

            def body():
                # per-image 1-row boundary tiles (block 63's B taps)
                x64 = []
                for n in range(nimg):
                    t64 = wpool.tile([16, w + 2], F32R, name=f"x64_{n}")
                    nc.vector.memset(t64[:].bitcast(F32), 0.0)
                    nc.gpsimd.dma_start(
                        out=t64[0:C, 1 : w + 1], in_=xr[n, h - 1 : h, :, :]
                    )
                    x64.append(t64)

                tiles = {}

                def load(g):
                    # global tile index g = n*nblocks + t, t in [0, nblocks)
                    n, t = divmod(g, nblocks)
                    xt = xpool.tile([128, w + 2], F32R, name="xt")
                    # cols 0 and w+1 are zero padding; fp32r isn't a DVE
                    # dtype, so memset via fp32 bitcast.
                    nc.vector.memset(xt[:, 0:1].bitcast(F32), 0.0)
                    nc.vector.memset(xt[:, w + 1 : w + 2].bitcast(F32), 0.0)
                    if t == 0:
                        # row -1 is zero padding (partitions 0..8)
                        nc.vector.memset(xt[0:C, :].bitcast(F32), 0.0)
                        nc.gpsimd.dma_start(
                            out=xt[C:128, 1 : w + 1], in_=xr[n, 0 : HB - 1, :, :]
                        )
                    else:
                        h0 = t * HB - 1
                        nc.gpsimd.dma_start(
                            out=xt[:, 1 : w + 1], in_=xr[n, h0 : h0 + HB, :, :]
                        )
                    tiles[g] = xt

                load(0)
                load(1)
                for g in range(nall):
                    n, t = divmod(g, nblocks)
                    if g + 2 < nall:
                        load(g + 2)
                    xa = tiles[g]
                    xb = tiles[g + 1] if t < nblocks - 1 else x64[n]
                    ps = pspool.tile([M, w], F32, name="ps", tag="ps")
                    # A taps (K=128, from tile t), then B taps (K=16,
                    # first 2 rows of tile t+1), accumulated per bank.
                    # Each stationary serves both w-chunks back to back.
                    for kw in range(KS):
                        for j in range(nchunks):
                            c0 = j * WCHUNK + kw
                            nc.tensor.matmul(
                                ps[:, j * WCHUNK : (j + 1) * WCHUNK],
                                wta[kw][:],
                                xa[:, c0 : c0 + WCHUNK],
                                start=(kw == 0),
                                stop=False,
                            )
                    for kw in range(KS):
                        for j in range(nchunks):
                            c0 = j * WCHUNK + kw
                            nc.tensor.matmul(
                                ps[:, j * WCHUNK : (j + 1) * WCHUNK],
                                wtb[kw][:],
                                xb[0:16, c0 : c0 + WCHUNK],
                                start=False,
                                stop=(kw == KS - 1),
                            )
                    del tiles[g]
                    ot = opool.tile([M, w], F32, name="ot", tag="ot")
                    nc.vector.tensor_scalar_add(ot[:], ps[:], bt[:])
                    # alternate output DMAs across both HWDGE rings; fan
                    # out over all three for the tail (input stream done)
                    if g >= nall - 8:
                        dma_eng = (nc.sync, nc.scalar, nc.gpsimd)[g % 3]
                    else:
                        dma_eng = nc.sync if g % 2 == 0 else nc.scalar
                    dma_eng.dma_start(
                        out=outr[n, t * HB : (t + 1) * HB, :, :],
                        in_=ot[:],
                    )

            # static unroll: tc.For_i loop control hits a walrus codegen
            # gap in this build ("ISA wrong length" on CompareAndBranch)
            for _ in range(reps):
                body()

    _split_excess_waits(nc)
    return nc


def _band_inputs(weight, bias):
    weight = np.asarray(weight, dtype=np.float32)
    bias = np.asarray(bias, dtype=np.float32)
    # S_full[kw][(r*8+ci), (ho*8+co)] = weight[co, ci, r-ho, kw]
    # r = input row - (16t-1) in [0, 18); out row = 16t + ho, ho in [0, 16)
    S = np.zeros((KS, HB + 2, C, HB, C), dtype=np.float32)
    for kw in range(KS):
        for ho in range(HB):
            for kh in range(KS):
                r = ho + kh
                S[kw, r, :, ho, :] = weight[:, :, kh, kw].T  # [ci, co]
    S = S.reshape(KS, (HB + 2) * C, M)
    SA = np.ascontiguousarray(S[:, :128])
    SB = np.ascontiguousarray(S[:, 128:144])
    biasv = np.tile(bias, HB).astype(np.float32)[:, None]  # m = ho*8 + co
    return SA, SB, biasv


def _in_maps(x, weight, bias, nimg_per_core, n_cores):
    SA, SB, biasv = _band_inputs(weight, bias)
    x = np.ascontiguousarray(x, dtype=np.float32)
    return [
        {
            "x": x[i * nimg_per_core : (i + 1) * nimg_per_core],
            "wa": SA,
            "wb": SB,
            "biasv": biasv,
        }
        for i in range(n_cores)
    ]


def _run(x, weight, bias, nimg_per_core, h, w, n_cores, reps=1):
    in_maps = _in_maps(x, weight, bias, nimg_per_core, n_cores)
    # The walrus backend compile is rarely flaky (parallel codegen race).
    # jax caches the failed compilation by HLO, so retries must change the
    # BIR bytes (salt) and drop the jit cache.
    last_exc = None
    for attempt in range(4):
        try:
            nc = _build(nimg_per_core, h, w, reps, salt=attempt)
            res = run_bass_kernel_spmd(nc, in_maps, core_ids=list(range(n_cores)))
            break
        except Exception as e:  # noqa: BLE001
            last_exc = e
            try:
                import jax

                jax.clear_caches()
            except Exception:  # noqa: BLE001
                pass
    else:
        raise last_exc
    return np.concatenate([r["out"] for r in res.results], axis=0)


def kernel(x, weight, bias):
    return _run(x, weight, bias, IMG_PER_CORE, H, W, N_CORES, reps=1)
